# revision 1
# baseline (speedup 1.0000x reference)
"""Trainium2 Bass kernel for nn_BertAttention_78554951843978.

Reference computation (B=2, S=2048, D=1024, H=16, hd=64, fp32):
    q = split_heads(hs @ Wq.T + bq); k = ...; v = ...
    probs = softmax(q k^T / sqrt(64)); ctx = probs @ v
    x = relu(merge_heads(ctx) + hs @ Wp.T)
    out = layernorm(x) * gamma + beta        (eps = 1e-12)

Sharding (8 cores): data-parallel over B (2 groups of 4 cores), tensor-
parallel over heads within a group (4 heads / 256 dims of D per core).
Each core computes, for its batch b and head block g:
    qT,kT  [256, 2048]  (transposed projections, W-stationary matmuls)
    v      [2048, 256]  (natural layout, hs-stationary) + ones column (aug)
    scoresT[ks, qs] per head -> exp on ACT -> pT
    ctxT accumulated via matmul with ones-augmented V  (row 64 = denom)
    x = relu(ctxT/denom + hs @ WpT); layernorm stats via ones-matmul
    partition reductions + AllReduce([4 cores]) of (sum, sumsq).
Output per core: outT [256, 2048] = out[b, :, g*256:(g+1)*256].T

Projection matmuls for head-pair 1 and the residual projection are
interleaved into the attention phase as PE filler work so the exp
pipeline (ACT) starts ~15us into the kernel.
"""

import numpy as np

import concourse.bass as bass
import concourse.tile as tile
from concourse import mybir
from concourse.bass_utils import run_bass_kernel_spmd

B, S, D, H = 2, 2048, 1024, 16
HD = 64
NCORES = 8
GROUPS = 4          # cores per batch
DC = D // GROUPS    # 256 dims per core
EPS = 1e-12

F32 = mybir.dt.float32
F32R = mybir.dt.float32r
AF = mybir.ActivationFunctionType


def _split_waits(nc, keep=1):
    """This container's walrus rejects >1 sem wait per (non-EVSEM)
    instruction ("Too many sync wait commands"); hoist extras onto
    preceding single-wait NOPs on the same engine."""
    for bb in nc.main_func.blocks:
        insts = list(bb.instructions)
        out_list = []
        changed = False
        for inst in insts:
            si = inst.sync_info
            cap = 2 if isinstance(inst, mybir.InstEventSemaphore) else keep
            if si is not None and si.on_wait is not None and len(si.on_wait) > cap:
                waits = list(si.on_wait)
                for w in waits[cap:]:
                    out_list.append(mybir.InstNoOp(
                        name=nc.get_next_instruction_name(),
                        engine=inst.engine,
                        ins=[], outs=[],
                        sync_info=mybir.SyncInfo(on_wait=[w], on_update=[]),
                        bass_nofuse=True,
                    ))
                inst.sync_info = mybir.SyncInfo(
                    on_wait=waits[:cap], on_update=list(si.on_update or []))
                changed = True
            out_list.append(inst)
        if changed:
            bb.instructions = out_list


def build_bass():
    nc = bass.Bass(num_devices=NCORES)

    # ---------------- DRAM I/O ----------------
    hsT_d = nc.dram_tensor("hsT", [D, S], F32R, kind="ExternalInput")
    wqT_d = nc.dram_tensor("wqT", [D, DC], F32R, kind="ExternalInput")
    wkT_d = nc.dram_tensor("wkT", [D, DC], F32R, kind="ExternalInput")
    wvT_d = nc.dram_tensor("wvT", [D, DC], F32R, kind="ExternalInput")
    wpT_d = nc.dram_tensor("wpT", [D, DC], F32R, kind="ExternalInput")
    bq_d = nc.dram_tensor("bq", [DC], F32, kind="ExternalInput")
    bk_d = nc.dram_tensor("bk", [DC], F32, kind="ExternalInput")
    bv_d = nc.dram_tensor("bv", [DC], F32, kind="ExternalInput")
    gm_d = nc.dram_tensor("gamma", [DC], F32, kind="ExternalInput")
    bt_d = nc.dram_tensor("beta", [DC], F32, kind="ExternalInput")
    out_d = nc.dram_tensor("outT", [DC, S], F32, kind="ExternalOutput")

    KT = D // 128    # 8 contraction tiles
    MT = DC // 128   # 2 output tiles of 128 dims (a head pair each)
    NS = S // 512    # 4 chunks of 512 along s
    ST = S // 128    # 16 key/sequence tiles of 128

    with tile.TileContext(nc) as tc:
        with (
            tc.tile_pool(name="persist", bufs=1) as persist,
            tc.tile_pool(name="dram", bufs=1, space="DRAM") as dram,
        ):
            # ------------- persistent SBUF -------------
            qT = persist.tile([128, MT, S], F32R)           # 16 KB/part
            kT = persist.tile([128, MT, S], F32R)
            x = persist.tile([128, MT, S], F32)             # res, then out
            vA = persist.tile([128, ST, GROUPS, HD + 1], F32R)  # aug V
            # consolidated small constants: cols 0..7 = bq|bk|gm|bt (2 each),
            # 8..71 = ones (fp32), 72..327 = bv broadcast
            cst = persist.tile([128, 8 + 64 + DC], F32)
            bq_s, bk_s = cst[:, 0:2], cst[:, 2:4]
            gm_s, bt_s = cst[:, 4:6], cst[:, 6:8]
            ones64 = cst[:, 8:72]
            bv_b = cst[:, 72:72 + DC]
            eps16 = cst[0:16, 8:9]   # reuse a ones col: set before, see below

            p1sb_cm = tc.tile_pool(name="p1sb", bufs=1)
            p1sb = p1sb_cm.__enter__()
            hsT = p1sb.tile([128, KT, S], F32R)             # 64 KB/part
            wq = p1sb.tile([128, KT, MT, 128], F32R)        # 8 KB/part each
            wk = p1sb.tile([128, KT, MT, 128], F32R)
            wv = p1sb.tile([128, KT, DC], F32R)
            wp = p1sb.tile([128, KT, MT, 128], F32R)

            # input DMAs: interleave so the k=0 tiles land first.
            dmae = [nc.sync, nc.scalar, nc.gpsimd, nc.scalar]
            hsT_t = hsT_d.rearrange("(t p) s -> p t s", p=128)
            wq_t = wqT_d.rearrange("(t p) (m f) -> p t m f", p=128, f=128)
            wk_t = wkT_d.rearrange("(t p) (m f) -> p t m f", p=128, f=128)
            wv_t = wvT_d.rearrange("(t p) c -> p t c", p=128)
            wp_t = wpT_d.rearrange("(t p) (m f) -> p t m f", p=128, f=128)
            for k in range(KT):
                e = dmae[k % 4]
                e.dma_start(out=hsT[:, k, :], in_=hsT_t[:, k, :])
                e.dma_start(out=wq[:, k], in_=wq_t[:, k])
                e.dma_start(out=wk[:, k], in_=wk_t[:, k])
            for k in range(KT):
                e = dmae[k % 4]
                e.dma_start(out=wp[:, k], in_=wp_t[:, k])
                e.dma_start(out=wv[:, k, :], in_=wv_t[:, k, :])
            nc.sync.dma_start(out=bq_s, in_=bq_d.rearrange("(m p) -> p m", p=128))
            nc.sync.dma_start(out=bk_s, in_=bk_d.rearrange("(m p) -> p m", p=128))
            nc.sync.dma_start(out=gm_s, in_=gm_d.rearrange("(m p) -> p m", p=128))
            nc.sync.dma_start(out=bt_s, in_=bt_d.rearrange("(m p) -> p m", p=128))
            nc.sync.dma_start(out=bv_b, in_=bass.AP(
                tensor=bv_d[:].tensor, offset=0, ap=[[0, 128], [1, DC]]))
            nc.vector.memset(ones64, 1.0)
            # ones column of the augmented V (gpsimd DMA: it may cast f32->f32r)
            nc.gpsimd.dma_start(
                out=vA[:, :, :, HD],
                in_=ones64.rearrange("p (s h) -> p s h", h=GROUPS))

            with (
                tc.tile_pool(name="pps", bufs=2, space="PSUM") as pps,
                tc.tile_pool(name="scps", bufs=2, space="PSUM") as scps,
                tc.tile_pool(name="ctxps", bufs=2, space="PSUM") as ctxps,
                tc.tile_pool(name="ptp", bufs=4) as ptp,
                tc.tile_pool(name="small", bufs=1) as small,
                tc.tile_pool(name="stg", bufs=2) as stg,
            ):
                def proj_group(w_sb, m, n, bias, out_sb):
                    """One [128,512] output block of a W-stationary projection."""
                    ps = pps.tile([128, 512], F32, name="gps")
                    for k in range(KT):
                        nc.tensor.matmul(
                            out=ps, lhsT=w_sb[:, k, m, :],
                            rhs=hsT[:, k, n * 512:(n + 1) * 512],
                            start=(k == 0), stop=(k == KT - 1))
                    o = out_sb[:, m, n * 512:(n + 1) * 512]
                    if bias is not None:
                        nc.vector.tensor_scalar_add(out=o, in0=ps, scalar1=bias)
                    else:
                        nc.vector.tensor_copy(out=o, in_=ps)

                def v_group(j):
                    """V (natural layout) for s-tile j, hs stationary."""
                    ps = pps.tile([128, 512], F32, name="gps")
                    for k in range(KT):
                        nc.tensor.matmul(
                            out=ps[:, 0:DC],
                            lhsT=hsT[:, k, j * 128:(j + 1) * 128],
                            rhs=wv[:, k, :],
                            start=(k == 0), stop=(k == KT - 1))
                    nc.vector.tensor_add(
                        out=vA[:, j, :, 0:HD],
                        in0=ps[:, 0:DC].rearrange("p (h d) -> p h d", d=HD),
                        in1=bv_b.rearrange("p (h d) -> p h d", d=HD))

                def g_q(m, n):
                    return lambda: proj_group(wq, m, n, bq_s[:, m:m + 1], qT)

                def g_k(m, n):
                    return lambda: proj_group(wk, m, n, bk_s[:, m:m + 1], kT)

                def g_r(m, n):
                    return lambda: proj_group(wp, m, n, None, x)

                # upfront: just enough for the first attention block
                for g in [g_q(0, 0), g_k(0, 0), g_k(0, 1), g_k(0, 2),
                          g_k(0, 3), g_r(0, 0)]:
                    g()
                # remaining projections run as PE filler work inside the
                # attention phase, ordered so each lands before its consumer
                fillers = [
                    # popped during hp0/qs1 (8 slots)
                    g_q(0, 1), g_r(0, 1), g_q(0, 2), g_r(0, 2),
                    g_q(0, 3), g_r(0, 3), g_q(1, 0), g_k(1, 0),
                    # popped during hp0/qs2 (8 slots)
                    g_k(1, 1), g_k(1, 2), g_k(1, 3), g_q(1, 1),
                    g_q(1, 2), g_q(1, 3), g_r(1, 0), g_r(1, 1),
                    # popped during hp0/qs3
                    g_r(1, 2), g_r(1, 3),
                ]
                fillers.reverse()   # consumed via pop()

                # ================= attention =================
                scr = dram.tile([MT * NS, 1024], F32)   # recip bounce
                for hp in range(MT):
                    for qn in range(NS):
                        qs = slice(qn * 512, (qn + 1) * 512)
                        ctx0 = ctxps.tile([128, 512], F32, name="ctx")
                        ctx1 = ctxps.tile([128, 512], F32, name="ctx")

                        def ctx_mms(pt, ks):
                            nc.tensor.matmul(
                                out=ctx0[0:HD + 1, :],
                                lhsT=vA[:, ks, 2 * hp, :],
                                rhs=pt[:, 0:512],
                                start=(ks == 0), stop=(ks == ST - 1))
                            nc.tensor.matmul(
                                out=ctx1[0:HD + 1, :],
                                lhsT=vA[:, ks, 2 * hp + 1, :],
                                rhs=pt[:, 512:1024],
                                start=(ks == 0), stop=(ks == ST - 1))

                        # software pipeline: ctx matmuls run one ks behind the
                        # scores/exp so the PE never waits on the current exp.
                        prev = None
                        for ks in range(ST):
                            if hp == 0 and qn == 0:
                                v_group(ks)
                            elif fillers and ks % 3 == 0:
                                fillers.pop()()
                            sc = scps.tile([128, 1024], F32, name="sc")
                            kslc = slice(ks * 128, (ks + 1) * 128)
                            nc.tensor.matmul(
                                out=sc[:, 0:512],
                                lhsT=kT[0:64, hp, kslc],
                                rhs=qT[0:64, hp, qs])
                            nc.tensor.matmul(
                                out=sc[:, 512:1024],
                                lhsT=kT[64:128, hp, kslc],
                                rhs=qT[64:128, hp, qs])
                            pt = ptp.tile([128, 1024], F32R, name="pt")
                            nc.scalar.activation(
                                out=pt, in_=sc, func=AF.Exp,
                                scale=float(1.0 / np.sqrt(HD)))
                            if prev is not None:
                                ctx_mms(*prev)
                            prev = (pt, ks)
                        ctx_mms(*prev)
                        # x[:, hp, qs] += ctx/denom  (x already holds res).
                        # Copy ctx PSUM -> SBUF first so the PSUM banks free
                        # up for the next block immediately.
                        ctxc = stg.tile([128, 1024], F32, name="ctxc")
                        nc.vector.tensor_copy(
                            out=ctxc[0:HD + 1, 0:512], in_=ctx0[0:HD + 1, :])
                        nc.vector.tensor_copy(
                            out=ctxc[0:HD + 1, 512:1024], in_=ctx1[0:HD + 1, :])
                        rout = small.tile([128, 1024], F32, name="rout")
                        nc.vector.reciprocal(
                            out=rout[64:65, 0:512], in_=ctxc[HD:HD + 1, 0:512])
                        nc.vector.reciprocal(
                            out=rout[64:65, 512:1024],
                            in_=ctxc[HD:HD + 1, 512:1024])
                        blk = hp * NS + qn
                        nc.sync.dma_start(
                            out=scr[blk:blk + 1, :], in_=rout[64:65, :])
                        rb = rout[0:64, :]   # bcast lands in unused partitions
                        nc.sync.dma_start(
                            out=rb,
                            in_=bass.AP(tensor=scr.tensor,
                                        offset=scr.offset + blk * 1024,
                                        ap=[[0, 64], [1, 1024]]))
                        tmp0 = stg.tile([64, 512], F32, name="tmp")
                        nc.vector.tensor_mul(
                            out=tmp0, in0=ctxc[0:64, 0:512], in1=rb[:, 0:512])
                        nc.vector.tensor_add(
                            out=x[0:64, hp, qs], in0=x[0:64, hp, qs], in1=tmp0)
                        tmp1 = stg.tile([64, 512], F32, name="tmp")
                        nc.vector.tensor_mul(
                            out=tmp1, in0=ctxc[0:64, 512:1024],
                            in1=rb[:, 512:1024])
                        s64 = stg.tile([128, 512], F32, name="s64")
                        nc.sync.dma_start(out=s64[64:128, :], in_=tmp1)
                        nc.vector.tensor_add(
                            out=x[64:128, hp, qs], in0=x[64:128, hp, qs],
                            in1=s64[64:128, :])
            p1sb_cm.__exit__(None, None, None)

            # ================= layernorm =================
            with (
                tc.tile_pool(name="p3ps", bufs=1, space="PSUM") as p3ps,
                tc.tile_pool(name="xrp", bufs=1) as xrp,
                tc.tile_pool(name="x2p", bufs=2) as x2p,
                tc.tile_pool(name="abp", bufs=1) as abp,
                tc.tile_pool(name="rows", bufs=1) as rows,
            ):
                nc.vector.memset(eps16, EPS)
                onesr = vA[:, 0, 0, HD:HD + 1]          # [128,1] fp32r ones
                # x = ctx/denom + res (pre-relu); xr = relu(x), rounded to
                # fp32r so the stats matmuls may consume it
                xr = xrp.tile([128, MT, S], F32R, name="xr")
                for t in range(MT):
                    nc.vector.tensor_scalar_max(
                        out=xr[:, t, :], in0=x[:, t, :], scalar1=0.0)
                sum_ps = p3ps.tile([1, S], F32, name="sum_ps")
                sq_ps = p3ps.tile([1, S], F32, name="sq_ps")
                x2t = []
                for t in range(MT):
                    x2 = x2p.tile([128, S], F32R, name="x2")
                    nc.scalar.activation(
                        out=x2, in_=xr[:, t, :].bitcast(F32), func=AF.Square)
                    x2t.append(x2)
                for n in range(NS):
                    ns = slice(n * 512, (n + 1) * 512)
                    for t in range(MT):
                        nc.tensor.matmul(
                            out=sum_ps[:, ns], lhsT=onesr,
                            rhs=xr[:, t, ns],
                            start=(t == 0), stop=(t == MT - 1))
                    for t in range(MT):
                        nc.tensor.matmul(
                            out=sq_ps[:, ns], lhsT=onesr, rhs=x2t[t][:, ns],
                            start=(t == 0), stop=(t == MT - 1))

                cc_in = dram.tile([1, 2 * S], F32)
                cc_out = dram.tile([1, 2 * S], F32)
                stats_sb = rows.tile([1, 2 * S], F32, name="stats_sb")
                nc.vector.tensor_copy(out=stats_sb[0:1, 0:S], in_=sum_ps)
                nc.vector.tensor_copy(out=stats_sb[0:1, S:2 * S], in_=sq_ps)
                nc.sync.dma_start(out=cc_in, in_=stats_sb)
                nc.gpsimd.collective_compute(
                    "AllReduce", mybir.AluOpType.add,
                    replica_groups=[[0, 1, 2, 3], [4, 5, 6, 7]],
                    ins=[cc_in.opt()], outs=[cc_out.opt()],
                )
                # row math on [16, 128] layout
                rsb = rows.tile([16, 2, 128], F32, name="rsb")
                nc.sync.dma_start(
                    out=rsb,
                    in_=cc_out.rearrange("r (v p f) -> p (r v) f", v=2, f=128))
                mean_sq = rows.tile([16, 128], F32, name="mean_sq")
                nc.vector.tensor_scalar_mul(
                    out=mean_sq, in0=rsb[:, 1, :], scalar1=1.0 / D)
                mu2 = rows.tile([16, 128], F32, name="mu2")
                nc.scalar.activation(
                    out=mu2, in_=rsb[:, 0, :], func=AF.Square, scale=1.0 / D)
                var = rows.tile([16, 128], F32, name="var")
                nc.vector.tensor_tensor(
                    out=var, in0=mean_sq, in1=mu2,
                    op=mybir.AluOpType.subtract)
                sd = rows.tile([16, 128], F32, name="sd")
                nc.scalar.activation(
                    out=sd, in_=var, func=AF.Sqrt, bias=eps16)
                rstd = rows.tile([16, 128], F32, name="rstd")
                nc.vector.reciprocal(out=rstd, in_=sd)
                mu = rows.tile([16, 128], F32, name="mu")
                nc.vector.tensor_scalar_mul(
                    out=mu, in0=rsb[:, 0, :], scalar1=1.0 / D)
                nB = rows.tile([16, 128], F32, name="nB")
                nc.vector.tensor_tensor(
                    out=nB, in0=mu, in1=rstd, op=mybir.AluOpType.mult)
                nc.vector.tensor_scalar_mul(out=nB, in0=nB, scalar1=-1.0)

                scr2 = dram.tile([2, S], F32)
                nc.sync.dma_start(
                    out=scr2[0:1, :].rearrange("r (p f) -> (r p) f", f=128),
                    in_=rstd)
                nc.sync.dma_start(
                    out=scr2[1:2, :].rearrange("r (p f) -> (r p) f", f=128),
                    in_=nB)
                Ab = abp.tile([128, S], F32, name="Ab")
                Bb = abp.tile([128, S], F32, name="Bb")
                nc.sync.dma_start(
                    out=Ab, in_=bass.AP(tensor=scr2.tensor, offset=scr2.offset,
                                        ap=[[0, 128], [1, S]]))
                nc.sync.dma_start(
                    out=Bb, in_=bass.AP(tensor=scr2.tensor,
                                        offset=scr2.offset + S,
                                        ap=[[0, 128], [1, S]]))
                out_t = out_d.rearrange("(t p) s -> p t s", p=128)
                # apply from full-precision relu(x) (xr is fp32r-rounded and
                # only used for the stats reductions); chunked along s so the
                # output DMA overlaps the remaining DVE passes
                for t in range(MT):
                    for n in range(2):
                        cs = slice(n * (S // 2), (n + 1) * (S // 2))
                        nc.vector.tensor_scalar_max(
                            out=x[:, t, cs], in0=x[:, t, cs], scalar1=0.0)
                        nc.vector.tensor_mul(
                            out=x[:, t, cs], in0=x[:, t, cs], in1=Ab[:, cs])
                        nc.vector.tensor_add(
                            out=x[:, t, cs], in0=x[:, t, cs], in1=Bb[:, cs])
                        nc.vector.tensor_scalar(
                            out=x[:, t, cs], in0=x[:, t, cs],
                            scalar1=gm_s[:, t:t + 1], scalar2=bt_s[:, t:t + 1],
                            op0=mybir.AluOpType.mult, op1=mybir.AluOpType.add)
                        nc.sync.dma_start(
                            out=out_t[:, t, cs], in_=x[:, t, cs])
    _split_waits(nc)
    return nc


_NC = None
LAST_RESULT = None


def _get_nc():
    global _NC
    if _NC is None:
        _NC = build_bass()
    return _NC


def kernel(hidden_states, Wq, bq, Wk, bk, Wv, bv, Wp, gamma, beta):
    hs = np.ascontiguousarray(np.asarray(hidden_states, dtype=np.float32))
    Wq = np.asarray(Wq, np.float32)
    Wk = np.asarray(Wk, np.float32)
    Wv = np.asarray(Wv, np.float32)
    Wp = np.asarray(Wp, np.float32)
    bq = np.asarray(bq, np.float32)
    bk = np.asarray(bk, np.float32)
    bv = np.asarray(bv, np.float32)
    gamma = np.asarray(gamma, np.float32)
    beta = np.asarray(beta, np.float32)

    nc = _get_nc()
    in_maps = []
    for c in range(NCORES):
        b, g = divmod(c, GROUPS)
        sl = slice(g * DC, (g + 1) * DC)
        in_maps.append({
            "hsT": np.ascontiguousarray(hs[b].T),
            "wqT": np.ascontiguousarray(Wq[sl].T),
            "wkT": np.ascontiguousarray(Wk[sl].T),
            "wvT": np.ascontiguousarray(Wv[sl].T),
            "wpT": np.ascontiguousarray(Wp[sl].T),
            "bq": np.ascontiguousarray(bq[sl]),
            "bk": np.ascontiguousarray(bk[sl]),
            "bv": np.ascontiguousarray(bv[sl]),
            "gamma": np.ascontiguousarray(gamma[sl]),
            "beta": np.ascontiguousarray(beta[sl]),
        })
    res = run_bass_kernel_spmd(nc, in_maps, core_ids=list(range(NCORES)))
    global LAST_RESULT
    LAST_RESULT = res
    out = np.empty((B, S, D), np.float32)
    for c, r in enumerate(res.results):
        b, g = divmod(c, GROUPS)
        out[b, :, g * DC:(g + 1) * DC] = r["outT"].T
    return out

